# revision 2
# baseline (speedup 1.0000x reference)
"""Mistral attention (B=1, S=2048, H=4096, 32 q-heads / 8 kv-heads GQA,
RoPE, causal) on 8 trn2 NeuronCores — v2.

Sharding: tensor-parallel by kv head. Core c owns kv head c, q heads
4c..4c+3, and Wo rows 512c..512c+512 (output column shard). Attention
outputs are AllGathered per 512-token chunk; each core then computes its
512-row slice of the output projection.

v2 changes vs v1:
- In-order chunk pipeline: for each 512-token chunk c: QKV projection ->
  RoPE -> attention (keys 0..c only, causal) -> AllGather(c). The AGs
  fire ~200us earlier and fully overlap compute; o-proj runs last.
- Softmax denominator: exp tiles are accumulated on DVE (fp16, 2x mode)
  into exsum; one ones^T@exsum matmul per head replaces the per-key-tile
  denominator matmuls (saves ~47us of PE), and the reciprocal broadcast
  uses a fp16 K=1 matmul.
- Value path (hs, weights, V, exp, attention out, AG payload) is fp16
  instead of bf16: same PE rate and bytes, 2 more mantissa bits.
- PSUM: 8 banks with explicit tags; scp0/scp1 (2 banks each) cycle
  through proj accumulators -> score pairs -> o-proj accumulators; t0/t1
  hold k/v accumulators then attention AV; t6/t7 hold V transposes, the
  denominator and its broadcast.
"""

import math

import ml_dtypes
import numpy as np

P = 128
S = 2048
H = 4096
HD = 128
NQH = 4  # q heads per core
TC = 512  # token chunk
NT = S // TC  # 4 chunks
HT = H // P  # 32 h tiles
N_CORES = 8
ROPE_THETA = 10000.0

_BUILT = None
F16 = ml_dtypes.float16 if hasattr(ml_dtypes, "float16") else np.float16


def _rope_tables():
    """cosT/sin2T in [hd partition, token free] layout.

    sin2T is the sin table pre-shifted/signed so that
    q_rot = q*cosT + shift128(q*sin2T), where shift128 swaps the two
    64-partition halves.
    """
    inv_freq = 1.0 / (ROPE_THETA ** (np.arange(0, HD, 2, dtype=np.float64) / HD))
    t = np.arange(S, dtype=np.float64)
    freqs = np.outer(t, inv_freq)  # [S, 64]
    emb = np.concatenate([freqs, freqs], axis=1)  # [S, HD]
    cosT = np.cos(emb).T.astype(np.float32)  # [HD, S]
    sinT = np.sin(emb).T.astype(np.float32)
    sin2T = sinT.copy()
    sin2T[64:] = -sin2T[64:]
    return (
        np.ascontiguousarray(cosT).astype(F16),
        np.ascontiguousarray(sin2T).astype(F16),
    )


def _masks():
    """4 diagonal-tile masks [128, 4*512] f16: mask_m[i, j] = (j >= i + m*128)."""
    i = np.arange(P)[:, None]
    j = np.arange(TC)[None, :]
    ms = [(j >= i + m * P).astype(np.float32) for m in range(4)]
    return np.ascontiguousarray(np.concatenate(ms, axis=1)).astype(F16)


def _build():
    import concourse.bacc as bacc
    import concourse.mybir as mybir
    import concourse.tile as tile

    f32 = mybir.dt.float32
    f32r = mybir.dt.float32r
    f16 = mybir.dt.float16

    nc = bacc.Bacc(
        "TRN2", target_bir_lowering=False, debug=False, num_devices=N_CORES
    )

    hsT = nc.declare_dram_parameter("hsT", [H, S], f16, isOutput=False)
    wqT = nc.declare_dram_parameter("wqT", [H, NQH * HD], f16, isOutput=False)
    wkT = nc.declare_dram_parameter("wkT", [H, HD], f16, isOutput=False)
    wvT = nc.declare_dram_parameter("wvT", [H, HD], f16, isOutput=False)
    woT = nc.declare_dram_parameter("woT", [H, NQH * HD], f16, isOutput=False)
    out_ext = nc.declare_dram_parameter("out", [NQH * HD, S], f16, isOutput=True)

    cosT_np, sin2T_np = _rope_tables()
    cos_dram = nc.inline_tensor(cosT_np, name="cosT")
    sin_dram = nc.inline_tensor(sin2T_np, name="sin2T")
    mask_dram = nc.inline_tensor(_masks(), name="masks")
    id_dram = nc.inline_tensor(np.eye(P).astype(F16), name="ident")

    ag_in = [nc.dram_tensor(f"ag_in{c}", [NQH * HD, TC], f16) for c in range(NT)]
    ag_out = [
        nc.dram_tensor(f"ag_out{c}", [N_CORES * NQH * HD, TC], f16, addr_space="Shared")
        for c in range(NT)
    ]

    Exp = mybir.ActivationFunctionType.Exp
    SCALE = 1.0 / math.sqrt(HD)

    with tile.TileContext(nc) as tc:
        with (
            tc.tile_pool(name="const", bufs=1) as constp,
            tc.tile_pool(name="qkvout", bufs=1) as qp,
            tc.tile_pool(name="pmain", bufs=1, space="PSUM") as pm,
            tc.tile_pool(name="wqkv", bufs=1) as wp,
            tc.tile_pool(name="hsp", bufs=5) as hsp,
            tc.tile_pool(name="work", bufs=2) as workp,
            tc.tile_pool(name="wo", bufs=1) as wop,
        ):
            # constants
            cos_sb = constp.tile([P, S], f16)
            sin_sb = constp.tile([P, S], f16)
            ones_sb = constp.tile([P, 1], f16)
            onesrow_sb = constp.tile([1, P], f32)
            nbias_sb = constp.tile([P, 1], f32)
            nc.gpsimd.memset(nbias_sb[:], -8.0)
            id_sb = constp.tile([P, P], f16)
            mask_sb = constp.tile([P, 4 * TC], f16)
            # constants go through SWDGE so the HWDGE queues start on the
            # hsT/weight stream immediately
            nc.gpsimd.dma_start(out=cos_sb[:], in_=cos_dram[:])
            nc.gpsimd.dma_start(out=sin_sb[:], in_=sin_dram[:])
            nc.gpsimd.memset(ones_sb[:], 1.0)
            nc.gpsimd.memset(onesrow_sb[:], 1.0)
            nc.gpsimd.dma_start(out=id_sb[:], in_=id_dram[:])
            nc.gpsimd.dma_start(out=mask_sb[:], in_=mask_dram[:])

            # persistent qkv outputs (fp16: 16-bit PE stream rate + enough
            # mantissa for the score path)
            qT_sb = qp.tile([P, NQH * S], f16)  # [hd, (head, t)]
            kT_sb = qp.tile([P, S], f16)
            vnat_sb = qp.tile([P, S], f16)  # [t%128, (ttile, hd)]

            wq_sb = wp.tile([P, HT * NQH * HD], f16)
            wk_sb = wp.tile([P, HT * HD], f16)
            wv_sb = wp.tile([P, HT * HD], f16)
            wo_sb = wop.tile([P, HT * NQH * HD], f16)
            wo_loaded = 0

            # batched weight loads: few big DMAs so sequencer issue slots
            # (~0.6us each) don't crowd out compute on the shared queues.
            def _load_wkv():
                nc.sync.dma_start(
                    out=wk_sb[:].rearrange("p (k t) -> p k t", k=HT),
                    in_=wkT[:].rearrange("(k p) t -> p k t", p=P),
                )
                nc.scalar.dma_start(
                    out=wv_sb[:].rearrange("p (k t) -> p k t", k=HT),
                    in_=wvT[:].rearrange("(k p) t -> p k t", p=P),
                )

            def _load_wq(ht):
                # one wq h-tile, just-in-time with the hsT stream
                eng = nc.scalar if ht % 2 == 0 else nc.sync
                eng.dma_start(
                    out=wq_sb[:, ht * 512 : (ht + 1) * 512],
                    in_=wqT[ht * P : (ht + 1) * P, :],
                )

            def _load_wo(blk):
                eng = nc.scalar if blk % 2 == 0 else nc.sync
                eng.dma_start(
                    out=wo_sb[:, blk * 8 * 512 : (blk + 1) * 8 * 512].rearrange(
                        "p (k t) -> p k t", k=8
                    ),
                    in_=woT[blk * 8 * P : (blk + 1) * 8 * P, :].rearrange(
                        "(k p) t -> p k t", p=P
                    ),
                )

            def bank(t, name):
                return pm.tile([P, TC], f32, tag=f"t{t}", bufs=1, name=name)

            def bank1(t, name):
                return pm.tile([1, TC], f32, tag=f"t{t}", bufs=1,
                               padded_shape=[P, TC], name=name)

            # ---- pipeline: per chunk c: proj -> RoPE -> attention -> AG ----
            for c in range(NT):
                # -- projection of chunk c --
                aq01 = pm.tile([P, 2 * TC], f32, tag="scp0", bufs=1,
                               name=f"aq01_{c}")
                aq23 = pm.tile([P, 2 * TC], f32, tag="scp1", bufs=1,
                               name=f"aq23_{c}")
                accs = [
                    aq01[:, 0:TC], aq01[:, TC : 2 * TC],
                    aq23[:, 0:TC], aq23[:, TC : 2 * TC],
                    bank(0, f"acck_{c}"), bank(1, f"accv_{c}"),
                ]

                def _lhsT(o, ht):
                    if o < 4:
                        return wq_sb[:, ht * 512 + o * P : ht * 512 + (o + 1) * P]
                    if o == 4:
                        return wk_sb[:, ht * P : (ht + 1) * P]
                    return wv_sb[:, ht * P : (ht + 1) * P]

                for htp in range(0, HT, 2):
                    # one DMA stages both h-tiles of the pair: [256, TC] DRAM
                    # -> [128, 2*TC] SBUF. Deep bufs ride out the ~10us DMA
                    # outages while each AllGather's mesh traffic drains.
                    # hsT stays off the scalar queue so ACT compute is never
                    # stuck behind DMA issue slots.
                    hst = hsp.tile([P, 2 * TC], f16, tag="hs", bufs=7)
                    nc.sync.dma_start(
                        out=hst[:].rearrange("p (k t) -> p k t", k=2),
                        in_=hsT[
                            htp * P : (htp + 2) * P, c * TC : (c + 1) * TC
                        ].rearrange("(k p) t -> p k t", p=P),
                    )
                    if c == 0 and htp == 0:
                        _load_wkv()
                    if c == 0:
                        _load_wq(htp)
                        _load_wq(htp + 1)
                    hsts = [hst[:, 0:TC], hst[:, TC : 2 * TC]]
                    # k and v first within each pair so their accumulators
                    # finish first at the end (eviction order below)
                    for o in (4, 5, 0, 1, 2, 3):
                        nc.tensor.matmul(
                            accs[o], _lhsT(o, htp), hsts[0],
                            start=(htp == 0), stop=False,
                        )
                        nc.tensor.matmul(
                            accs[o], _lhsT(o, htp + 1), hsts[1],
                            start=False, stop=(htp + 1 == HT - 1),
                        )

                # -- eviction + RoPE --
                # v: ACT copy out of psum, then PE transposes
                vtmp = workp.tile([P, TC], f16, tag="vtmp")
                nc.scalar.copy(vtmp[:], accs[5])
                for j in range(4):
                    tp = pm.tile([P, P], f16, tag=f"t{6 + j % 2}", bufs=1,
                                 padded_shape=[P, TC], name=f"vt_{c}_{j}")
                    nc.tensor.transpose(tp[:], vtmp[:, j * P : (j + 1) * P], id_sb[:])
                    nc.vector.tensor_copy(
                        vnat_sb[:, (c * 4 + j) * P : (c * 4 + j + 1) * P], tp[:]
                    )

                # q0 first: head 0's ascending score MMs touch old-chunk keys
                # first, so only q0's RoPE gates the first matmul. ACT stages
                # each accumulator to fp16 so the DVE chain runs in 2x mode.
                for o in (0, 4, 1, 2, 3):
                    acc = accs[o]
                    if o < 4:
                        dst = qT_sb[:, o * S + c * TC : o * S + (c + 1) * TC]
                    else:
                        dst = kT_sb[:, c * TC : (c + 1) * TC]
                    st = workp.tile([P, TC], f16, tag="ropest")
                    nc.scalar.copy(st[:], acc)
                    # u = shift128(q * sin2): write the halves partition-shifted
                    u = workp.tile([P, TC], f16, tag="ropes")
                    w = workp.tile([P, TC], f16, tag="ropec")
                    sslc = sin_sb[:, c * TC : (c + 1) * TC]
                    nc.vector.tensor_mul(u[64:128, :], st[0:64, :], sslc[0:64, :])
                    nc.vector.tensor_mul(u[0:64, :], st[64:128, :], sslc[64:128, :])
                    nc.vector.tensor_mul(
                        w[:], st[:], cos_sb[:, c * TC : (c + 1) * TC]
                    )
                    nc.vector.tensor_add(dst[:], w[:], u[:])

                # -- attention for chunk c (keys 0..c, causal diagonal) --
                nkt = 4 * c + 4
                for h in range(NQH):
                    av = bank((c * 4 + h) % 2, f"av_{c}_{h}")
                    # ascending kts for the first head of the chunk (gives
                    # DVE time to finish the new chunk's V-transpose copies
                    # and RoPE); descending for the rest so the masked
                    # diagonal exp chain hides behind the unmasked stream.
                    if h == 0:
                        kts = list(range(nkt))
                    else:
                        kts = list(range(nkt - 1, -1, -1))
                    first_kt, last_kt = kts[0], kts[-1]
                    pairs = [(kts[i], kts[i + 1]) for i in range(0, nkt, 2)]
                    exsum = workp.tile([P, 2 * TC], f16, tag="exsum", bufs=2,
                                       name=f"exsum_{c}_{h}")
                    for pi, (ka, kb) in enumerate(pairs):
                        scp = pm.tile(
                            [P, 2 * TC], f32, tag=f"scp{pi % 2}", bufs=1,
                            name=f"scp_{c}_{h}_{pi}",
                        )
                        for half, kt in ((0, ka), (1, kb)):
                            nc.tensor.matmul(
                                scp[:, half * TC : (half + 1) * TC],
                                kT_sb[:, kt * P : (kt + 1) * P],
                                qT_sb[:, h * S + c * TC : h * S + (c + 1) * TC],
                                start=True,
                                stop=True,
                            )
                        ex = workp.tile([P, 2 * TC], f16, tag="exp", bufs=3,
                                        name=f"ex_{c}_{h}_{pi}")
                        # bias -8 keeps exp(score) in fp16 range (max score
                        # ~16.6); the e^-8 factor cancels in normalization.
                        nc.scalar.activation(ex[:], scp[:], Exp, scale=SCALE,
                                             bias=nbias_sb[:])
                        for half, kt in ((0, ka), (1, kb)):
                            m = kt - 4 * c
                            if m >= 0:
                                nc.vector.tensor_mul(
                                    ex[:, half * TC : (half + 1) * TC],
                                    ex[:, half * TC : (half + 1) * TC],
                                    mask_sb[:, m * TC : (m + 1) * TC],
                                )
                        for half, kt in ((0, ka), (1, kb)):
                            nc.tensor.matmul(
                                av[:],
                                vnat_sb[:, kt * P : (kt + 1) * P],
                                ex[:, half * TC : (half + 1) * TC],
                                start=(kt == first_kt),
                                stop=(kt == last_kt),
                            )
                        # fp16 exp-sum accumulation (DVE 2x mode), one wide
                        # add per pair; halves are folded by the dn matmuls
                        if pi == 0:
                            nc.vector.tensor_copy(exsum[:], ex[:])
                        else:
                            nc.vector.tensor_add(exsum[:], exsum[:], ex[:])
                    # denominator: ones^T @ exsum halves -> [1, TC]
                    dn = bank1(6, f"dn_{c}_{h}")
                    nc.tensor.matmul(dn[:], ones_sb[:], exsum[:, 0:TC],
                                     start=True, stop=False)
                    nc.tensor.matmul(dn[:], ones_sb[:], exsum[:, TC : 2 * TC],
                                     start=False, stop=True)
                    rc = workp.tile([1, TC], f32, tag="rc")
                    nc.vector.reciprocal_approx_fast(rc[:], dn[:])
                    bcb = bank(7, f"bc_{c}_{h}")
                    nc.tensor.matmul(
                        bcb[:], onesrow_sb[:], rc[:], start=True, stop=True
                    )
                    avs = workp.tile([P, TC], f32, tag="avs", bufs=2)
                    nc.vector.tensor_copy(avs[:], av[:])
                    ao = workp.tile([P, TC], f16, tag="ao", bufs=4)
                    nc.vector.tensor_mul(ao[:], avs[:], bcb[:])
                    aow = nc.sync.dma_start(
                        out=ag_in[c][h * P : (h + 1) * P, :], in_=ao[:]
                    )
                    if c == NT - 1 and h == NQH - 2:
                        guard_aow = aow
                    if h == 0:
                        _load_wo(c)

                nc.gpsimd.collective_compute(
                    "AllGather",
                    mybir.AluOpType.bypass,
                    ins=[ag_in[c][:]],
                    outs=[ag_out[c][:]],
                    replica_groups=[list(range(N_CORES))],
                )

            # ---- o-proj: chunk order follows the AG completions ----
            for c in range(NT):
                if c % 2 == 0:
                    y01 = pm.tile([P, 2 * TC], f32, tag="scp0", bufs=1,
                                  name=f"y01_{c}")
                    y23 = pm.tile([P, 2 * TC], f32, tag="scp1", bufs=1,
                                  name=f"y23_{c}")
                    ys = [y01[:, 0:TC], y01[:, TC : 2 * TC],
                          y23[:, 0:TC], y23[:, TC : 2 * TC]]
                else:
                    ys = [bank(0, f"y0_{c}")[:], bank(1, f"y1_{c}")[:],
                          bank(6, f"y2_{c}")[:], bank(7, f"y3_{c}")[:]]
                for ot in range(HT):
                    agt = workp.tile([P, TC], f16, tag="ag", bufs=10)
                    eng = nc.sync if ot % 2 == 0 else nc.scalar
                    rd = eng.dma_start(
                        out=agt[:], in_=ag_out[c][ot * P : (ot + 1) * P, :]
                    )
                    if ot < 2:
                        # Keep o-proj DRAM reads behind the attention
                        # ag_in writes in the in-order HWDGE queues —
                        # a hoisted read would head-of-line block the
                        # hsT loads behind it while its AG completes.
                        tile.add_dep_helper(
                            rd.ins, guard_aow.ins,
                            reason="o-proj reads behind attention writes",
                        )
                    for yt in range(4):
                        nc.tensor.matmul(
                            ys[yt],
                            wo_sb[:, ot * 512 + yt * P : ot * 512 + (yt + 1) * P],
                            agt[:],
                            start=(ot == 0),
                            stop=(ot == HT - 1),
                        )
                for yt in range(4):
                    yo = workp.tile([P, TC], f16, tag="yo")
                    nc.scalar.copy(yo[:], ys[yt])
                    nc.sync.dma_start(
                        out=out_ext[yt * P : (yt + 1) * P, c * TC : (c + 1) * TC],
                        in_=yo[:],
                    )

    nc.finalize()
    return nc


def _get_built():
    global _BUILT
    if _BUILT is None:
        _BUILT = _build()
    return _BUILT


def make_in_maps(hidden_states, Wq, Wk, Wv, Wo):
    hs = np.asarray(hidden_states, dtype=np.float32).reshape(S, H)
    hsT = np.ascontiguousarray(hs.T).astype(F16)
    in_maps = []
    for c in range(N_CORES):
        in_maps.append(
            {
                "hsT": hsT,
                "wqT": np.ascontiguousarray(np.asarray(Wq)[c * 512 : (c + 1) * 512].T).astype(F16),
                "wkT": np.ascontiguousarray(np.asarray(Wk)[c * 128 : (c + 1) * 128].T).astype(F16),
                "wvT": np.ascontiguousarray(np.asarray(Wv)[c * 128 : (c + 1) * 128].T).astype(F16),
                "woT": np.ascontiguousarray(np.asarray(Wo)[c * 512 : (c + 1) * 512].T).astype(F16),
            }
        )
    return in_maps


def kernel(hidden_states, Wq, Wk, Wv, Wo):
    from concourse.bass_utils import run_bass_kernel_spmd

    nc = _get_built()
    in_maps = make_in_maps(hidden_states, Wq, Wk, Wv, Wo)
    r = run_bass_kernel_spmd(nc, in_maps, list(range(N_CORES)))
    yT = np.concatenate([r.results[c]["out"] for c in range(N_CORES)], axis=0)
    return np.ascontiguousarray(yT.T).reshape(1, S, H).astype(np.float32)


# revision 3
# speedup vs baseline: 1.0141x; 1.0141x over previous
"""Mistral attention (B=1, S=2048, H=4096, 32 q-heads / 8 kv-heads GQA,
RoPE, causal) on 8 trn2 NeuronCores — v2.

Sharding: tensor-parallel by kv head. Core c owns kv head c, q heads
4c..4c+3, and Wo rows 512c..512c+512 (output column shard). Attention
outputs are AllGathered per 512-token chunk; each core then computes its
512-row slice of the output projection.

v2 changes vs v1:
- In-order chunk pipeline: for each 512-token chunk c: QKV projection ->
  RoPE -> attention (keys 0..c only, causal) -> AllGather(c). The AGs
  fire ~200us earlier and fully overlap compute; o-proj runs last.
- Softmax denominator: exp tiles are accumulated on DVE (fp16, 2x mode)
  into exsum; one ones^T@exsum matmul per head replaces the per-key-tile
  denominator matmuls (saves ~47us of PE), and the reciprocal broadcast
  uses a fp16 K=1 matmul.
- Value path (hs, weights, V, exp, attention out, AG payload) is fp16
  instead of bf16: same PE rate and bytes, 2 more mantissa bits.
- PSUM: 8 banks with explicit tags; scp0/scp1 (2 banks each) cycle
  through proj accumulators -> score pairs -> o-proj accumulators; t0/t1
  hold k/v accumulators then attention AV; t6/t7 hold V transposes, the
  denominator and its broadcast.
"""

import math

import ml_dtypes
import numpy as np

P = 128
S = 2048
H = 4096
HD = 128
NQH = 4  # q heads per core
TC = 512  # token chunk
NT = S // TC  # 4 chunks
HT = H // P  # 32 h tiles
N_CORES = 8
ROPE_THETA = 10000.0

_BUILT = None
F16 = ml_dtypes.float16 if hasattr(ml_dtypes, "float16") else np.float16


def _rope_tables():
    """cosT/sin2T in [hd partition, token free] layout.

    sin2T is the sin table pre-shifted/signed so that
    q_rot = q*cosT + shift128(q*sin2T), where shift128 swaps the two
    64-partition halves.
    """
    inv_freq = 1.0 / (ROPE_THETA ** (np.arange(0, HD, 2, dtype=np.float64) / HD))
    t = np.arange(S, dtype=np.float64)
    freqs = np.outer(t, inv_freq)  # [S, 64]
    emb = np.concatenate([freqs, freqs], axis=1)  # [S, HD]
    cosT = np.cos(emb).T.astype(np.float32)  # [HD, S]
    sinT = np.sin(emb).T.astype(np.float32)
    sin2T = sinT.copy()
    sin2T[64:] = -sin2T[64:]
    return (
        np.ascontiguousarray(cosT).astype(F16),
        np.ascontiguousarray(sin2T).astype(F16),
    )


def _masks():
    """4 diagonal-tile masks [128, 4*512] f16: mask_m[i, j] = (j >= i + m*128)."""
    i = np.arange(P)[:, None]
    j = np.arange(TC)[None, :]
    ms = [(j >= i + m * P).astype(np.float32) for m in range(4)]
    return np.ascontiguousarray(np.concatenate(ms, axis=1)).astype(F16)


def _build():
    import concourse.bacc as bacc
    import concourse.mybir as mybir
    import concourse.tile as tile

    f32 = mybir.dt.float32
    f32r = mybir.dt.float32r
    f16 = mybir.dt.float16

    nc = bacc.Bacc(
        "TRN2", target_bir_lowering=False, debug=False, num_devices=N_CORES
    )

    hsT = nc.declare_dram_parameter("hsT", [H, S], f16, isOutput=False)
    wqT = nc.declare_dram_parameter("wqT", [H, NQH * HD], f16, isOutput=False)
    wkT = nc.declare_dram_parameter("wkT", [H, HD], f16, isOutput=False)
    wvT = nc.declare_dram_parameter("wvT", [H, HD], f16, isOutput=False)
    woT = nc.declare_dram_parameter("woT", [H, NQH * HD], f16, isOutput=False)
    out_ext = nc.declare_dram_parameter("out", [NQH * HD, S], f16, isOutput=True)

    cosT_np, sin2T_np = _rope_tables()
    cos_dram = nc.inline_tensor(cosT_np, name="cosT")
    sin_dram = nc.inline_tensor(sin2T_np, name="sin2T")
    mask_dram = nc.inline_tensor(_masks(), name="masks")
    id_dram = nc.inline_tensor(np.eye(P).astype(F16), name="ident")

    ag_in = [nc.dram_tensor(f"ag_in{c}", [NQH * HD, TC], f16) for c in range(NT)]
    ag_out = [
        nc.dram_tensor(f"ag_out{c}", [N_CORES * NQH * HD, TC], f16, addr_space="Shared")
        for c in range(NT)
    ]

    Exp = mybir.ActivationFunctionType.Exp
    SCALE = 1.0 / math.sqrt(HD)

    with tile.TileContext(nc) as tc:
        with (
            tc.tile_pool(name="const", bufs=1) as constp,
            tc.tile_pool(name="qkvout", bufs=1) as qp,
            tc.tile_pool(name="pmain", bufs=1, space="PSUM") as pm,
            tc.tile_pool(name="wqkv", bufs=1) as wp,
            tc.tile_pool(name="hsp", bufs=5) as hsp,
            tc.tile_pool(name="work", bufs=2) as workp,
            tc.tile_pool(name="wo", bufs=1) as wop,
        ):
            # constants
            cos_sb = constp.tile([P, S], f16)
            sin_sb = constp.tile([P, S], f16)
            ones_sb = constp.tile([P, 1], f16)
            onesrow_sb = constp.tile([1, P], f32)
            nbias_sb = constp.tile([P, 1], f32)
            nc.gpsimd.memset(nbias_sb[:], -8.0)
            id_sb = constp.tile([P, P], f16)
            mask_sb = constp.tile([P, 4 * TC], f16)
            # constants go through SWDGE so the HWDGE queues start on the
            # hsT/weight stream immediately
            nc.gpsimd.dma_start(out=cos_sb[:], in_=cos_dram[:])
            nc.gpsimd.dma_start(out=sin_sb[:], in_=sin_dram[:])
            nc.gpsimd.memset(ones_sb[:], 1.0)
            nc.gpsimd.memset(onesrow_sb[:], 1.0)
            nc.gpsimd.dma_start(out=id_sb[:], in_=id_dram[:])
            nc.gpsimd.dma_start(out=mask_sb[:], in_=mask_dram[:])

            # persistent qkv outputs (fp16: 16-bit PE stream rate + enough
            # mantissa for the score path)
            qT_sb = qp.tile([P, NQH * S], f16)  # [hd, (head, t)]
            kT_sb = qp.tile([P, S], f16)
            vnat_sb = qp.tile([P, S], f16)  # [t%128, (ttile, hd)]

            wq_sb = wp.tile([P, HT * NQH * HD], f16)
            wk_sb = wp.tile([P, HT * HD], f16)
            wv_sb = wp.tile([P, HT * HD], f16)
            wo_sb = wop.tile([P, HT * NQH * HD], f16)
            wo_loaded = 0

            # batched weight loads: few big DMAs so sequencer issue slots
            # (~0.6us each) don't crowd out compute on the shared queues.
            def _load_wkv():
                nc.sync.dma_start(
                    out=wk_sb[:].rearrange("p (k t) -> p k t", k=HT),
                    in_=wkT[:].rearrange("(k p) t -> p k t", p=P),
                )
                nc.scalar.dma_start(
                    out=wv_sb[:].rearrange("p (k t) -> p k t", k=HT),
                    in_=wvT[:].rearrange("(k p) t -> p k t", p=P),
                )

            def _load_wq(ht):
                # one wq h-tile, just-in-time with the hsT stream
                eng = nc.scalar if ht % 2 == 0 else nc.sync
                eng.dma_start(
                    out=wq_sb[:, ht * 512 : (ht + 1) * 512],
                    in_=wqT[ht * P : (ht + 1) * P, :],
                )

            def _load_wo(blk):
                eng = nc.scalar if blk % 2 == 0 else nc.sync
                eng.dma_start(
                    out=wo_sb[:, blk * 8 * 512 : (blk + 1) * 8 * 512].rearrange(
                        "p (k t) -> p k t", k=8
                    ),
                    in_=woT[blk * 8 * P : (blk + 1) * 8 * P, :].rearrange(
                        "(k p) t -> p k t", p=P
                    ),
                )

            def bank(t, name):
                return pm.tile([P, TC], f32, tag=f"t{t}", bufs=1, name=name)

            def bank1(t, name):
                return pm.tile([1, TC], f32, tag=f"t{t}", bufs=1,
                               padded_shape=[P, TC], name=name)

            # ---- pipeline: per chunk c: proj -> RoPE -> attention -> AG ----
            for c in range(NT):
                # -- projection of chunk c --
                aq01 = pm.tile([P, 2 * TC], f32, tag="scp0", bufs=1,
                               name=f"aq01_{c}")
                aq23 = pm.tile([P, 2 * TC], f32, tag="scp1", bufs=1,
                               name=f"aq23_{c}")
                accs = [
                    aq01[:, 0:TC], aq01[:, TC : 2 * TC],
                    aq23[:, 0:TC], aq23[:, TC : 2 * TC],
                    bank(0, f"acck_{c}"), bank(1, f"accv_{c}"),
                ]

                def _lhsT(o, ht):
                    if o < 4:
                        return wq_sb[:, ht * 512 + o * P : ht * 512 + (o + 1) * P]
                    if o == 4:
                        return wk_sb[:, ht * P : (ht + 1) * P]
                    return wv_sb[:, ht * P : (ht + 1) * P]

                for htp in range(0, HT, 2):
                    # one DMA stages both h-tiles of the pair: [256, TC] DRAM
                    # -> [128, 2*TC] SBUF. Deep bufs ride out the ~10us DMA
                    # outages while each AllGather's mesh traffic drains.
                    # hsT stays off the scalar queue so ACT compute is never
                    # stuck behind DMA issue slots.
                    hst = hsp.tile([P, 2 * TC], f16, tag="hs", bufs=7)
                    nc.sync.dma_start(
                        out=hst[:].rearrange("p (k t) -> p k t", k=2),
                        in_=hsT[
                            htp * P : (htp + 2) * P, c * TC : (c + 1) * TC
                        ].rearrange("(k p) t -> p k t", p=P),
                    )
                    if c == 0 and htp == 0:
                        _load_wkv()
                    if c == 0:
                        _load_wq(htp)
                        _load_wq(htp + 1)
                    hsts = [hst[:, 0:TC], hst[:, TC : 2 * TC]]
                    # k and v first within each pair so their accumulators
                    # finish first at the end (eviction order below)
                    for o in (4, 5, 0, 1, 2, 3):
                        nc.tensor.matmul(
                            accs[o], _lhsT(o, htp), hsts[0],
                            start=(htp == 0), stop=False,
                        )
                        nc.tensor.matmul(
                            accs[o], _lhsT(o, htp + 1), hsts[1],
                            start=False, stop=(htp + 1 == HT - 1),
                        )

                # -- eviction + RoPE --
                # v: ACT copy out of psum, then PE transposes
                vtmp = workp.tile([P, TC], f16, tag="vtmp")
                nc.scalar.copy(vtmp[:], accs[5])
                for j in range(4):
                    tp = pm.tile([P, P], f16, tag=f"t{6 + j % 2}", bufs=1,
                                 padded_shape=[P, TC], name=f"vt_{c}_{j}")
                    nc.tensor.transpose(tp[:], vtmp[:, j * P : (j + 1) * P], id_sb[:])
                    nc.vector.tensor_copy(
                        vnat_sb[:, (c * 4 + j) * P : (c * 4 + j + 1) * P], tp[:]
                    )

                # q0 first: head 0's ascending score MMs touch old-chunk keys
                # first, so only q0's RoPE gates the first matmul. ACT stages
                # each accumulator to fp16 so the DVE chain runs in 2x mode.
                for o in (0, 4, 1, 2, 3):
                    acc = accs[o]
                    if o < 4:
                        dst = qT_sb[:, o * S + c * TC : o * S + (c + 1) * TC]
                    else:
                        dst = kT_sb[:, c * TC : (c + 1) * TC]
                    st = workp.tile([P, TC], f16, tag="ropest")
                    nc.scalar.copy(st[:], acc)
                    # u = shift128(q * sin2): write the halves partition-shifted
                    u = workp.tile([P, TC], f16, tag="ropes")
                    w = workp.tile([P, TC], f16, tag="ropec")
                    sslc = sin_sb[:, c * TC : (c + 1) * TC]
                    nc.vector.tensor_mul(u[64:128, :], st[0:64, :], sslc[0:64, :])
                    nc.vector.tensor_mul(u[0:64, :], st[64:128, :], sslc[64:128, :])
                    nc.vector.tensor_mul(
                        w[:], st[:], cos_sb[:, c * TC : (c + 1) * TC]
                    )
                    nc.vector.tensor_add(dst[:], w[:], u[:])

                # -- attention for chunk c (keys 0..c, causal diagonal) --
                nkt = 4 * c + 4
                for h in range(NQH):
                    av = bank((c * 4 + h) % 2, f"av_{c}_{h}")
                    # ascending kts for the first head of the chunk (gives
                    # DVE time to finish the new chunk's V-transpose copies
                    # and RoPE); descending for the rest so the masked
                    # diagonal exp chain hides behind the unmasked stream.
                    if h == 0:
                        kts = list(range(nkt))
                    else:
                        kts = list(range(nkt - 1, -1, -1))
                    first_kt, last_kt = kts[0], kts[-1]
                    pairs = [(kts[i], kts[i + 1]) for i in range(0, nkt, 2)]
                    exsum = workp.tile([P, 2 * TC], f16, tag="exsum", bufs=2,
                                       name=f"exsum_{c}_{h}")
                    for pi, (ka, kb) in enumerate(pairs):
                        scp = pm.tile(
                            [P, 2 * TC], f32, tag=f"scp{pi % 2}", bufs=1,
                            name=f"scp_{c}_{h}_{pi}",
                        )
                        # causal trim: diagonal tile m only attends queries
                        # j >= m*128, so scores/exp/mask run on the suffix
                        # and the fully-masked prefix is zero-filled on the
                        # (otherwise idle) GpSimd engine.
                        trims = []
                        for half, kt in ((0, ka), (1, kb)):
                            m = kt - 4 * c
                            trims.append(m * P if m >= 1 else 0)
                        for (half, kt), tr in zip(((0, ka), (1, kb)), trims):
                            nc.tensor.matmul(
                                scp[:, half * TC + tr : (half + 1) * TC],
                                kT_sb[:, kt * P : (kt + 1) * P],
                                qT_sb[:, h * S + c * TC + tr : h * S + (c + 1) * TC],
                                start=True,
                                stop=True,
                            )
                        ex = workp.tile([P, 2 * TC], f16, tag="exp", bufs=3,
                                        name=f"ex_{c}_{h}_{pi}")
                        # bias -8 keeps exp(score) in fp16 range (max score
                        # ~16.6); the e^-8 factor cancels in normalization.
                        if trims[0] == 0 and trims[1] == 0:
                            nc.scalar.activation(ex[:], scp[:], Exp,
                                                 scale=SCALE, bias=nbias_sb[:])
                        else:
                            for half, tr in ((0, trims[0]), (1, trims[1])):
                                if tr > 0:
                                    nc.gpsimd.memset(
                                        ex[:, half * TC : half * TC + tr], 0.0
                                    )
                                nc.scalar.activation(
                                    ex[:, half * TC + tr : (half + 1) * TC],
                                    scp[:, half * TC + tr : (half + 1) * TC],
                                    Exp, scale=SCALE, bias=nbias_sb[:],
                                )
                        for (half, kt), tr in zip(((0, ka), (1, kb)), trims):
                            m = kt - 4 * c
                            if m >= 0:
                                nc.vector.tensor_mul(
                                    ex[:, half * TC + tr : (half + 1) * TC],
                                    ex[:, half * TC + tr : (half + 1) * TC],
                                    mask_sb[:, m * TC + tr : (m + 1) * TC],
                                )
                        for half, kt in ((0, ka), (1, kb)):
                            nc.tensor.matmul(
                                av[:],
                                vnat_sb[:, kt * P : (kt + 1) * P],
                                ex[:, half * TC : (half + 1) * TC],
                                start=(kt == first_kt),
                                stop=(kt == last_kt),
                            )
                        # fp16 exp-sum accumulation (DVE 2x mode), one wide
                        # add per pair; halves are folded by the dn matmuls
                        if pi == 0:
                            nc.vector.tensor_copy(exsum[:], ex[:])
                        else:
                            nc.vector.tensor_add(exsum[:], exsum[:], ex[:])
                    # denominator: ones^T @ exsum halves -> [1, TC]
                    dn = bank1(6, f"dn_{c}_{h}")
                    nc.tensor.matmul(dn[:], ones_sb[:], exsum[:, 0:TC],
                                     start=True, stop=False)
                    nc.tensor.matmul(dn[:], ones_sb[:], exsum[:, TC : 2 * TC],
                                     start=False, stop=True)
                    rc = workp.tile([1, TC], f32, tag="rc")
                    nc.vector.reciprocal_approx_fast(rc[:], dn[:])
                    bcb = bank(7, f"bc_{c}_{h}")
                    nc.tensor.matmul(
                        bcb[:], onesrow_sb[:], rc[:], start=True, stop=True
                    )
                    avs = workp.tile([P, TC], f32, tag="avs", bufs=2)
                    nc.vector.tensor_copy(avs[:], av[:])
                    ao = workp.tile([P, TC], f16, tag="ao", bufs=4)
                    nc.vector.tensor_mul(ao[:], avs[:], bcb[:])
                    aow = nc.sync.dma_start(
                        out=ag_in[c][h * P : (h + 1) * P, :], in_=ao[:]
                    )
                    if c == NT - 1 and h == NQH - 2:
                        guard_aow = aow
                    if h == 0:
                        _load_wo(c)

                nc.gpsimd.collective_compute(
                    "AllGather",
                    mybir.AluOpType.bypass,
                    ins=[ag_in[c][:]],
                    outs=[ag_out[c][:]],
                    replica_groups=[list(range(N_CORES))],
                )

            # ---- o-proj: chunk order follows the AG completions ----
            for c in range(NT):
                if c % 2 == 0:
                    y01 = pm.tile([P, 2 * TC], f32, tag="scp0", bufs=1,
                                  name=f"y01_{c}")
                    y23 = pm.tile([P, 2 * TC], f32, tag="scp1", bufs=1,
                                  name=f"y23_{c}")
                    ys = [y01[:, 0:TC], y01[:, TC : 2 * TC],
                          y23[:, 0:TC], y23[:, TC : 2 * TC]]
                else:
                    ys = [bank(0, f"y0_{c}")[:], bank(1, f"y1_{c}")[:],
                          bank(6, f"y2_{c}")[:], bank(7, f"y3_{c}")[:]]
                for ot in range(HT):
                    agt = workp.tile([P, TC], f16, tag="ag", bufs=10)
                    eng = nc.sync if ot % 2 == 0 else nc.scalar
                    rd = eng.dma_start(
                        out=agt[:], in_=ag_out[c][ot * P : (ot + 1) * P, :]
                    )
                    if ot < 2:
                        # Keep o-proj DRAM reads behind the attention
                        # ag_in writes in the in-order HWDGE queues —
                        # a hoisted read would head-of-line block the
                        # hsT loads behind it while its AG completes.
                        tile.add_dep_helper(
                            rd.ins, guard_aow.ins,
                            reason="o-proj reads behind attention writes",
                        )
                    for yt in range(4):
                        nc.tensor.matmul(
                            ys[yt],
                            wo_sb[:, ot * 512 + yt * P : ot * 512 + (yt + 1) * P],
                            agt[:],
                            start=(ot == 0),
                            stop=(ot == HT - 1),
                        )
                for yt in range(4):
                    yo = workp.tile([P, TC], f16, tag="yo")
                    nc.scalar.copy(yo[:], ys[yt])
                    nc.sync.dma_start(
                        out=out_ext[yt * P : (yt + 1) * P, c * TC : (c + 1) * TC],
                        in_=yo[:],
                    )

    nc.finalize()
    return nc


def _get_built():
    global _BUILT
    if _BUILT is None:
        _BUILT = _build()
    return _BUILT


def make_in_maps(hidden_states, Wq, Wk, Wv, Wo):
    hs = np.asarray(hidden_states, dtype=np.float32).reshape(S, H)
    hsT = np.ascontiguousarray(hs.T).astype(F16)
    in_maps = []
    for c in range(N_CORES):
        in_maps.append(
            {
                "hsT": hsT,
                "wqT": np.ascontiguousarray(np.asarray(Wq)[c * 512 : (c + 1) * 512].T).astype(F16),
                "wkT": np.ascontiguousarray(np.asarray(Wk)[c * 128 : (c + 1) * 128].T).astype(F16),
                "wvT": np.ascontiguousarray(np.asarray(Wv)[c * 128 : (c + 1) * 128].T).astype(F16),
                "woT": np.ascontiguousarray(np.asarray(Wo)[c * 512 : (c + 1) * 512].T).astype(F16),
            }
        )
    return in_maps


def kernel(hidden_states, Wq, Wk, Wv, Wo):
    from concourse.bass_utils import run_bass_kernel_spmd

    nc = _get_built()
    in_maps = make_in_maps(hidden_states, Wq, Wk, Wv, Wo)
    r = run_bass_kernel_spmd(nc, in_maps, list(range(N_CORES)))
    yT = np.concatenate([r.results[c]["out"] for c in range(N_CORES)], axis=0)
    return np.ascontiguousarray(yT.T).reshape(1, S, H).astype(np.float32)
